# revision 19
# baseline (speedup 1.0000x reference)
"""Causal self-attention 2d (B=8, C=256, H=W=32, kdim=out_ch=512, 8 heads)
on 8 TRN2 NeuronCores, data-parallel over batch (1 batch element / core).

Per-core math (batch element b); projections in float32r, attention
operands (qT/kT/es/va/tri) in bf16 (ATTN_BF16), which dodges the fp32r
4x rate penalty on sub-256-column matmul tiles and double-rates the
DVE mask multiply. Input DMAs are split across the SP and ACT hwdge
rings (DMA_SPLIT); PV trails QK/exp by PV_LAG=4 chunks:
  xT   = input[b].reshape(C, S)                    # (256, 1024), channels-major
  qT   = q_w @ xT + q_b[:, None]                   # (512, 1024)  kdim-major
  kT   = k_w @ xT + k_b[:, None]
  v    = xT.T @ v_w.T + v_b                        # (1024, 512)  seq-major
  per head h (hd=64):
    ST[ki,qi] = (kT_h.T q ... ) = k_h @ q_h.T / 8  # transposed scores
    E = exp(ST) * causal(ki<=qi)
    PV: out_aug[dv+1, qi] = [v_h | 1].T @ E        # row 64 = softmax denom
    out_h = out_aug[:64] / denom                   # == (attn @ v_h).T
  out[b] = concat_h(out_h)                         # (512, 1024) = (out_ch, H*W)
"""

import numpy as np

import concourse.bass as bass
import concourse.tile as tile
from concourse import bacc, mybir
from concourse.bass_utils import run_bass_kernel_spmd

B, C, H, W = 8, 256, 32, 32
S = H * W            # 1024 sequence positions
KD = 512             # kdim == out_ch
NH = 8
HD = 64              # head dim (k and v)
NC_CORES = 8
F32 = mybir.dt.float32
F32R = mybir.dt.float32r
BF16 = mybir.dt.bfloat16
P = 128              # partitions
CK = C // P          # 2  contraction chunks for projections
MK = KD // P         # 4  kdim chunks
SK = S // P          # 8  seq chunks


MASK_ENGINE = "vector"   # "gpsimd" | "vector"
EVICT_BCAST = "dma"      # "gpsimd" | "dma" (DRAM-bounce broadcast)
EVICT_MUL = "vector"     # "gpsimd" | "vector" (pv * recip normalize mul)
DMA_SPLIT = True         # input DMAs split across SP + ACT hwdge rings
ATTN_BF16 = True         # attention operands (qT/kT/es/va/tri) in bf16
TIMING_MODE = None       # None | "dma_only" | "no_dma" (hw_loop diagnostics)
STAGE = 6                # 1 proj, 2 +qk, 3 +exp, 4 +mask, 5 +pv, 6 full
QK_WIDE = False          # qk psum tiles [P,1024] + one exp/chunk vs [P,512] + exp/range
BUFS = (2, 3, 3)         # (proj, qk, pv) pool sizes; banks: proj+ (2 if wide else 1)*qk + pv <= 8
V_IN_PAIR0 = True        # stream v projection inside pair 0's chunk loop
PV_LAG = 6               # chunks of lookahead between QK/exp and PV


def _emit(tc, reps=1, hw_loop=0):
    nc = tc.nc
    xT = nc.dram_tensor("xT", [C, S], F32R, kind="ExternalInput").ap()
    wqT = nc.dram_tensor("wqT", [C, KD], F32R, kind="ExternalInput").ap()
    wkT = nc.dram_tensor("wkT", [C, KD], F32R, kind="ExternalInput").ap()
    wvT = nc.dram_tensor("wvT", [C, NH * (HD + 1)], F32R, kind="ExternalInput").ap()
    qb = nc.dram_tensor("qb", [KD], F32, kind="ExternalInput").ap()
    kb = nc.dram_tensor("kb", [KD], F32, kind="ExternalInput").ap()
    vb = nc.dram_tensor("vb", [NH * (HD + 1)], F32R, kind="ExternalInput").ap()
    tri = nc.dram_tensor("tri", [P, P], BF16 if ATTN_BF16 else F32,
                         kind="ExternalInput").ap()
    ones = nc.dram_tensor("ones", [P, NH + 1], F32R, kind="ExternalInput").ap()
    out = nc.dram_tensor("out", [KD, S], F32, kind="ExternalOutput").ap()
    args = (xT, wqT, wkT, wvT, qb, kb, vb, tri, ones)
    if hw_loop:
        if TIMING_MODE == "no_dma":
            persist, tiles = _alloc_load(tc, *args)
            with tc.For_i(0, hw_loop, 1):
                _compute(tc, tiles, out)
            persist.release()
        else:
            with tc.For_i(0, hw_loop, 1):
                persist, tiles = _alloc_load(tc, *args)
                _compute(tc, tiles, out, dma_only=(TIMING_MODE == "dma_only"))
                persist.release()
    else:
        for _ in range(reps):
            persist, tiles = _alloc_load(tc, *args)
            _compute(tc, tiles, out)
            persist.release()


def _alloc_load(tc, xT, wqT, wkT, wvT, qb, kb, vb, tri, ones):
    nc = tc.nc
    persist = tc.alloc_tile_pool(name="persist", bufs=1)

    # ---- persistent SBUF tensors -------------------------------------
    xT_sb = [persist.tile([P, S], F32R, tag=f"xT{i}", name=f"xT{i}") for i in range(CK)]
    wq_sb = [persist.tile([P, KD], F32R, tag=f"wq{i}", name=f"wq{i}") for i in range(CK)]
    wk_sb = [persist.tile([P, KD], F32R, tag=f"wk{i}", name=f"wk{i}") for i in range(CK)]
    wv_sb = [persist.tile([P, NH * (HD + 1)], F32R, tag=f"wv{i}", name=f"wv{i}")
             for i in range(CK)]
    ADT = BF16 if ATTN_BF16 else F32R
    TRI_DT = BF16 if ATTN_BF16 else F32
    qT_sb = [persist.tile([P, S], ADT, tag=f"qT{i}", name=f"qT{i}") for i in range(MK)]
    kT_sb = [persist.tile([P, S], ADT, tag=f"kT{i}", name=f"kT{i}") for i in range(MK)]
    # v augmented: per seq-chunk, cols 65h..65h+63 = v_h, col 65h+64 = 1.0
    va_sb = [persist.tile([P, NH * (HD + 1)], ADT, tag=f"va{i}", name=f"va{i}") for i in range(SK)]
    # exp(S^T) per head-in-pair (2 sets), per ki-chunk, double-buffered by
    # pair parity so pair i+1's exp doesn't wait on pair i's trailing PV
    es_sb = [[[persist.tile([P, S], ADT, tag=f"es{p}_{j}_{i}",
                            name=f"es{p}_{j}_{i}") for i in range(SK)]
              for j in range(2)] for p in range(2)]
    out_sb = [persist.tile([P, S], F32, tag=f"out{i}", name=f"out{i}") for i in range(MK)]
    qb_sb = persist.tile([P, MK], F32, tag="qb")
    kb_sb = persist.tile([P, MK], F32, tag="kb")
    vb_sb = persist.tile([1, NH * (HD + 1)], F32R, tag="vb")
    tri_sb = persist.tile([P, P], TRI_DT, tag="tri")
    # ones[:, 0] as a column feeding the K=1 v-bias matmul (lhsT [1, 128])
    ones_sb = persist.tile([1, P], F32R, tag="ones")

    # ---- input DMAs (ordered so the first projections start early) ---
    # With DMA_SPLIT, x + wk go on the ACT hwdge ring, the rest on SP's,
    # halving the serialized input-load stream.
    alt = nc.scalar if DMA_SPLIT else nc.sync
    for i in range(CK):
        nc.sync.dma_start(wq_sb[i][:], wqT[P * i:P * (i + 1), :])
        alt.dma_start(xT_sb[i][:], xT[P * i:P * (i + 1), :])
    for i in range(CK):
        alt.dma_start(wk_sb[i][:], wkT[P * i:P * (i + 1), :])
    nc.sync.dma_start(qb_sb[:], qb.rearrange("(c p) -> p c", p=P))
    nc.sync.dma_start(kb_sb[:], kb.rearrange("(c p) -> p c", p=P))
    for i in range(CK):
        nc.sync.dma_start(wv_sb[i][:], wvT[P * i:P * (i + 1), :])
    nc.sync.dma_start(vb_sb[:], vb.rearrange("(a d) -> a d", a=1))
    alt.dma_start(tri_sb[:], tri[:, :])
    nc.sync.dma_start(ones_sb[:], ones[:, 0].rearrange("(a p) -> a p", a=1))

    return persist, dict(
        xT_sb=xT_sb, wq_sb=wq_sb, wk_sb=wk_sb, wv_sb=wv_sb, qT_sb=qT_sb,
        kT_sb=kT_sb, va_sb=va_sb, es_sb=es_sb, out_sb=out_sb, qb_sb=qb_sb,
        kb_sb=kb_sb, vb_sb=vb_sb, tri_sb=tri_sb, ones_sb=ones_sb)


def _compute(tc, t, out, dma_only=False):
    nc = tc.nc
    mask_eng = nc.gpsimd if MASK_ENGINE == "gpsimd" else nc.vector
    (xT_sb, wq_sb, wk_sb, wv_sb, qT_sb, kT_sb, va_sb, es_sb, out_sb, qb_sb,
     kb_sb, vb_sb, tri_sb, ones_sb) = (
        t["xT_sb"], t["wq_sb"], t["wk_sb"], t["wv_sb"], t["qT_sb"], t["kT_sb"],
        t["va_sb"], t["es_sb"], t["out_sb"], t["qb_sb"], t["kb_sb"], t["vb_sb"],
        t["tri_sb"], t["ones_sb"])
    small = tc.alloc_tile_pool(name="small", bufs=8)
    if dma_only:
        nc.sync.dma_start(out[0:P, 0:S], xT_sb[0][:].bitcast(F32))
        small.release()
        return

    # ---- fused projections + attention ------------------------------
    bp, bq, bv = BUFS
    assert bp + (2 if QK_WIDE else 1) * bq + bv <= 8
    dramp = tc.alloc_tile_pool(name="dramp", bufs=4, space="DRAM")
    with (
        tc.tile_pool(name="proj_psum", bufs=bp, space="PSUM") as pp,
        tc.tile_pool(name="qk_psum", bufs=bq, space="PSUM") as qkp,
        tc.tile_pool(name="pv_psum", bufs=bv, space="PSUM") as pvp,
    ):
        # warm the ACT exp table while input DMAs run
        warm = small.tile([1, 4], F32, tag="warm", name="warm")
        nc.scalar.activation(warm[:], ones_sb[0:1, 0:4].bitcast(F32),
                             mybir.ActivationFunctionType.Exp)

        def proj_qk_chunk(i):
            for (w_sb, dst, b_sb) in ((wq_sb, qT_sb, qb_sb), (wk_sb, kT_sb, kb_sb)):
                for half in range(2):
                    o = 512 * half
                    ps = pp.tile([P, 512], F32, tag="proj", name="proj")
                    for kc in range(CK):
                        nc.tensor.matmul(
                            ps[:], w_sb[kc][:, P * i:P * (i + 1)],
                            xT_sb[kc][:, o:o + 512],
                            start=(kc == 0), stop=(kc == CK - 1))
                    nc.vector.tensor_scalar_add(
                        dst[i][:, o:o + 512], ps[:], b_sb[:, i:i + 1])

        HW2 = NH * (HD + 1) // 2  # 260 aug-columns per half

        def proj_v_chunk(s):
            for half in range(2):
                o = HW2 * half
                ps = pp.tile([P, HW2], F32, tag="proj", name="proj")
                for kc in range(CK):
                    nc.tensor.matmul(
                        ps[:], xT_sb[kc][:, P * s:P * (s + 1)],
                        wv_sb[kc][:, o:o + HW2],
                        start=(kc == 0), stop=False)
                nc.tensor.matmul(
                    ps[:], ones_sb[0:1, :], vb_sb[0:1, o:o + HW2],
                    start=False, stop=True)
                nc.vector.tensor_copy(va_sb[s][:, o:o + HW2], ps[:])

        def evict(i, hp, rr, t):
            """normalize pv psum tile t -> out_sb slice -> DRAM (frees t)."""
            o = 512 * rr
            rc = small.tile([1, 512], F32, tag="recip", name="recip")
            nc.vector.reciprocal(rc[:], t[HD:HD + 1, :])
            bc = small.tile([HD, 512], F32, tag="bc", name="bc")
            if EVICT_BCAST == "dma":
                rd = dramp.tile([1, 512], F32, tag="rd", name="rd")
                nc.sync.dma_start(rd[:], rc[:])
                rd_b = bass.AP(tensor=rd.tensor, offset=rd.offset,
                               ap=[[0, HD]] + list(rd.ap)[1:])
                nc.sync.dma_start(bc[:], rd_b)
            else:
                nc.gpsimd.partition_broadcast(bc[:], rc[:])
            mul_eng = nc.gpsimd if EVICT_MUL == "gpsimd" else nc.vector
            mul_eng.tensor_mul(
                out_sb[i][HD * hp:HD * (hp + 1), o:o + 512],
                t[0:HD, :], bc[:])
            nc.sync.dma_start(
                out[P * i + HD * hp:P * i + HD * (hp + 1), o:o + 512],
                out_sb[i][HD * hp:HD * (hp + 1), o:o + 512])

        proj_qk_chunk(0)
        if not V_IN_PAIR0:
            for s in range(SK):
                proj_v_chunk(s)

        cmax = {0: CK * 2 - 1, 1: SK - 1}  # last ki-chunk per qi-range

        def emit_qk_exp(i, c):
            if STAGE < 2:
                return
            sc = P * c
            es = es_sb[i % 2]
            ranges = [rr for rr in range(2) if sc <= 512 * rr + 511]
            if QK_WIDE:
                qk = [qkp.tile([P, S], F32, tag="qk", name="qk")
                      for _ in range(2)]
                for rr in ranges:
                    o = 512 * rr
                    lo = max(o, sc)
                    for hp in range(2):
                        ro = HD * hp
                        nc.tensor.matmul(
                            qk[hp][:, lo:o + 512],
                            kT_sb[i][ro:ro + HD, sc:sc + P],
                            qT_sb[i][ro:ro + HD, lo:o + 512],
                            start=True, stop=True, tile_position=(ro, 0))
                for hp in range(2):
                    if STAGE < 3:
                        continue
                    nc.scalar.activation(
                        es[hp][c][:, sc:S], qk[hp][:, sc:S],
                        mybir.ActivationFunctionType.Exp, scale=0.125)
                    if STAGE >= 4:
                        mask_eng.tensor_mul(
                            es[hp][c][:, sc:sc + P],
                            es[hp][c][:, sc:sc + P], tri_sb[:])
            else:
                for rr in ranges:
                    o = 512 * rr
                    lo = max(o, sc)
                    qk = [qkp.tile([P, 512], F32, tag="qk", name="qk")
                          for _ in range(2)]
                    for hp in range(2):
                        ro = HD * hp
                        nc.tensor.matmul(
                            qk[hp][:, lo - o:512],
                            kT_sb[i][ro:ro + HD, sc:sc + P],
                            qT_sb[i][ro:ro + HD, lo:o + 512],
                            start=True, stop=True, tile_position=(ro, 0))
                    for hp in range(2):
                        if STAGE < 3:
                            continue
                        nc.scalar.activation(
                            es[hp][c][:, lo:o + 512],
                            qk[hp][:, lo - o:512],
                            mybir.ActivationFunctionType.Exp, scale=0.125)
                        if rr == c // 4 and STAGE >= 4:
                            mask_eng.tensor_mul(
                                es[hp][c][:, sc:sc + P],
                                es[hp][c][:, sc:sc + P], tri_sb[:])

        pv_tiles = {}

        def emit_pv(i, c):
            if STAGE < 5:
                return
            if i not in pv_tiles:
                pv_tiles[i] = [[pvp.tile([P, 512], F32, tag="pv", name="pv")
                                for _ in range(2)] for _ in range(2)]
            pv = pv_tiles[i]
            sc = P * c
            ranges = [rr for rr in range(2) if sc <= 512 * rr + 511]
            for rr in ranges:
                o = 512 * rr
                lo = max(o, sc)
                for hp in range(2):
                    h = 2 * i + hp
                    nc.tensor.matmul(
                        pv[hp][rr][0:HD + 1, lo - o:512],
                        va_sb[c][:, (HD + 1) * h:(HD + 1) * (h + 1)],
                        es_sb[i % 2][hp][c][:, lo:o + 512],
                        start=(c == 0), stop=(c == cmax[rr]))
                if c == cmax[rr] and STAGE >= 6:
                    for hp in range(2):
                        evict(i, hp, rr, pv[hp][rr])

        LA = PV_LAG
        seq = [(i, c) for i in range(MK) for c in range(SK)]
        for idx, (i, c) in enumerate(seq):
            if i == 0 and V_IN_PAIR0:
                proj_v_chunk(c)  # stream v in during the first pair
            if c == 3 and i + 1 < MK:
                proj_qk_chunk(i + 1)
            emit_qk_exp(i, c)
            if idx >= LA:
                emit_pv(*seq[idx - LA])
        for idx in range(len(seq) - LA, len(seq)):
            emit_pv(*seq[idx])

    dramp.release()
    small.release()


_NC = None


def build_nc(reps=1, hw_loop=0):
    nc = bacc.Bacc("TRN2", target_bir_lowering=False, debug=False,
                   num_devices=NC_CORES)
    with tile.TileContext(nc) as tc:
        _emit(tc, reps=reps, hw_loop=hw_loop)
    nc.compile()
    return nc


def _get_nc():
    global _NC
    if _NC is None:
        _NC = build_nc()
    return _NC


def make_in_maps(input, q_w, q_b, k_w, k_b, v_w, v_b):
    xT_all = np.ascontiguousarray(input.reshape(B, C, S), dtype=np.float32)
    wqT = np.ascontiguousarray(q_w.T, dtype=np.float32)
    wkT = np.ascontiguousarray(k_w.T, dtype=np.float32)
    # v weights augmented with a zero column per head whose bias is 1.0 ->
    # the projection emits [v_h | 1] directly (softmax denominator column)
    wv = np.asarray(v_w, dtype=np.float32).reshape(NH, HD, C)
    wv_aug = np.concatenate([wv, np.zeros((NH, 1, C), np.float32)], axis=1)
    wvT = np.ascontiguousarray(wv_aug.reshape(NH * (HD + 1), C).T)
    vb_aug = np.concatenate(
        [np.asarray(v_b, np.float32).reshape(NH, HD),
         np.ones((NH, 1), np.float32)], axis=1).reshape(-1)
    tri = np.triu(np.ones((P, P), dtype=np.float32))  # 1 where ki<=qi
    if ATTN_BF16:
        import ml_dtypes
        tri = tri.astype(ml_dtypes.bfloat16)
    shared = {
        "wqT": wqT, "wkT": wkT, "wvT": wvT,
        "qb": np.ascontiguousarray(q_b, np.float32),
        "kb": np.ascontiguousarray(k_b, np.float32),
        "vb": vb_aug,
        "tri": tri,
        "ones": np.ones((P, NH + 1), dtype=np.float32),
    }
    return [{"xT": xT_all[b], **shared} for b in range(B)]


def kernel(input, q_w, q_b, k_w, k_b, v_w, v_b):
    nc = _get_nc()
    in_maps = make_in_maps(input, q_w, q_b, k_w, k_b, v_w, v_b)
    res = run_bass_kernel_spmd(nc, in_maps, core_ids=list(range(NC_CORES)))
    out = np.stack([res.results[b]["out"] for b in range(B)])
    return out.reshape(B, KD, H, W)



# revision 21
# speedup vs baseline: 4.6489x; 4.6489x over previous
"""Causal self-attention 2d (B=8, C=256, H=W=32, kdim=out_ch=512, 8 heads)
on 8 TRN2 NeuronCores, data-parallel over batch (1 batch element / core).

Per-core math (batch element b); projections in float32r, attention
operands (qT/kT/es/va/tri) in bf16 (ATTN_BF16), which dodges the fp32r
4x rate penalty on sub-256-column matmul tiles and double-rates the
DVE mask multiply. Input DMAs are split across the SP and ACT hwdge
rings (DMA_SPLIT); PV trails QK/exp by PV_LAG=4 chunks:
  xT   = input[b].reshape(C, S)                    # (256, 1024), channels-major
  qT   = q_w @ xT + q_b[:, None]                   # (512, 1024)  kdim-major
  kT   = k_w @ xT + k_b[:, None]
  v    = xT.T @ v_w.T + v_b                        # (1024, 512)  seq-major
  per head h (hd=64):
    ST[ki,qi] = (kT_h.T q ... ) = k_h @ q_h.T / 8  # transposed scores
    E = exp(ST) * causal(ki<=qi)
    PV: out_aug[dv+1, qi] = [v_h | 1].T @ E        # row 64 = softmax denom
    out_h = out_aug[:64] / denom                   # == (attn @ v_h).T
  out[b] = concat_h(out_h)                         # (512, 1024) = (out_ch, H*W)
"""

import numpy as np

import concourse.bass as bass
import concourse.tile as tile
from concourse import bacc, mybir
from concourse.bass_utils import run_bass_kernel_spmd

B, C, H, W = 8, 256, 32, 32
S = H * W            # 1024 sequence positions
KD = 512             # kdim == out_ch
NH = 8
HD = 64              # head dim (k and v)
NC_CORES = 8
F32 = mybir.dt.float32
F32R = mybir.dt.float32r
BF16 = mybir.dt.bfloat16
P = 128              # partitions
CK = C // P          # 2  contraction chunks for projections
MK = KD // P         # 4  kdim chunks
SK = S // P          # 8  seq chunks


MASK_ENGINE = "vector"   # "gpsimd" | "vector"
EVICT_BCAST = "dma"      # "gpsimd" | "dma" (DRAM-bounce broadcast)
EVICT_MUL = "vector"     # "gpsimd" | "vector" (pv * recip normalize mul)
DMA_SPLIT = True         # input DMAs split across SP + ACT hwdge rings
ATTN_BF16 = True         # attention operands (qT/kT/es/va/tri) in bf16
TIMING_MODE = None       # None | "dma_only" | "no_dma" (hw_loop diagnostics)
STAGE = 6                # 1 proj, 2 +qk, 3 +exp, 4 +mask, 5 +pv, 6 full
QK_WIDE = False          # qk psum tiles [P,1024] + one exp/chunk vs [P,512] + exp/range
BUFS = (2, 3, 3)         # (proj, qk, pv) pool sizes; banks: proj+ (2 if wide else 1)*qk + pv <= 8
V_IN_PAIR0 = True        # stream v projection inside pair 0's chunk loop
PV_LAG = 5               # chunks of lookahead between QK/exp and PV


def _emit(tc, reps=1, hw_loop=0):
    nc = tc.nc
    xT = nc.dram_tensor("xT", [C, S], F32R, kind="ExternalInput").ap()
    wqT = nc.dram_tensor("wqT", [C, KD], F32R, kind="ExternalInput").ap()
    wkT = nc.dram_tensor("wkT", [C, KD], F32R, kind="ExternalInput").ap()
    wvT = nc.dram_tensor("wvT", [C, NH * (HD + 1)], F32R, kind="ExternalInput").ap()
    qb = nc.dram_tensor("qb", [KD], F32, kind="ExternalInput").ap()
    kb = nc.dram_tensor("kb", [KD], F32, kind="ExternalInput").ap()
    vb = nc.dram_tensor("vb", [NH * (HD + 1)], F32R, kind="ExternalInput").ap()
    tri = nc.dram_tensor("tri", [P, P], BF16 if ATTN_BF16 else F32,
                         kind="ExternalInput").ap()
    ones = nc.dram_tensor("ones", [P, NH + 1], F32R, kind="ExternalInput").ap()
    out = nc.dram_tensor("out", [KD, S], F32, kind="ExternalOutput").ap()
    args = (xT, wqT, wkT, wvT, qb, kb, vb, tri, ones)
    if hw_loop:
        if TIMING_MODE == "no_dma":
            persist, tiles = _alloc_load(tc, *args)
            with tc.For_i(0, hw_loop, 1):
                _compute(tc, tiles, out)
            persist.release()
        else:
            with tc.For_i(0, hw_loop, 1):
                persist, tiles = _alloc_load(tc, *args)
                _compute(tc, tiles, out, dma_only=(TIMING_MODE == "dma_only"))
                persist.release()
    else:
        for _ in range(reps):
            persist, tiles = _alloc_load(tc, *args)
            _compute(tc, tiles, out)
            persist.release()


def _alloc_load(tc, xT, wqT, wkT, wvT, qb, kb, vb, tri, ones):
    nc = tc.nc
    persist = tc.alloc_tile_pool(name="persist", bufs=1)

    # ---- persistent SBUF tensors -------------------------------------
    xT_sb = [persist.tile([P, S], F32R, tag=f"xT{i}", name=f"xT{i}") for i in range(CK)]
    wq_sb = [persist.tile([P, KD], F32R, tag=f"wq{i}", name=f"wq{i}") for i in range(CK)]
    wk_sb = [persist.tile([P, KD], F32R, tag=f"wk{i}", name=f"wk{i}") for i in range(CK)]
    wv_sb = [persist.tile([P, NH * (HD + 1)], F32R, tag=f"wv{i}", name=f"wv{i}")
             for i in range(CK)]
    ADT = BF16 if ATTN_BF16 else F32R
    TRI_DT = BF16 if ATTN_BF16 else F32
    qT_sb = [persist.tile([P, S], ADT, tag=f"qT{i}", name=f"qT{i}") for i in range(MK)]
    kT_sb = [persist.tile([P, S], ADT, tag=f"kT{i}", name=f"kT{i}") for i in range(MK)]
    # v augmented: per seq-chunk, cols 65h..65h+63 = v_h, col 65h+64 = 1.0
    va_sb = [persist.tile([P, NH * (HD + 1)], ADT, tag=f"va{i}", name=f"va{i}") for i in range(SK)]
    # exp(S^T) per head-in-pair (2 sets), per ki-chunk
    es_sb = [[persist.tile([P, S], ADT, tag=f"es{j}_{i}", name=f"es{j}_{i}") for i in range(SK)]
             for j in range(2)]
    out_sb = [persist.tile([P, S], F32, tag=f"out{i}", name=f"out{i}") for i in range(MK)]
    qb_sb = persist.tile([P, MK], F32, tag="qb")
    kb_sb = persist.tile([P, MK], F32, tag="kb")
    vb_sb = persist.tile([1, NH * (HD + 1)], F32R, tag="vb")
    tri_sb = persist.tile([P, P], TRI_DT, tag="tri")
    # ones[:, 0] as a column feeding the K=1 v-bias matmul (lhsT [1, 128])
    ones_sb = persist.tile([1, P], F32R, tag="ones")

    # ---- input DMAs (ordered so the first projections start early) ---
    # With DMA_SPLIT, x + wk go on the ACT hwdge ring, the rest on SP's,
    # halving the serialized input-load stream.
    alt = nc.scalar if DMA_SPLIT else nc.sync
    for i in range(CK):
        nc.sync.dma_start(wq_sb[i][:], wqT[P * i:P * (i + 1), :])
        alt.dma_start(xT_sb[i][:], xT[P * i:P * (i + 1), :])
    for i in range(CK):
        alt.dma_start(wk_sb[i][:], wkT[P * i:P * (i + 1), :])
    nc.sync.dma_start(qb_sb[:], qb.rearrange("(c p) -> p c", p=P))
    nc.sync.dma_start(kb_sb[:], kb.rearrange("(c p) -> p c", p=P))
    for i in range(CK):
        nc.sync.dma_start(wv_sb[i][:], wvT[P * i:P * (i + 1), :])
    nc.sync.dma_start(vb_sb[:], vb.rearrange("(a d) -> a d", a=1))
    alt.dma_start(tri_sb[:], tri[:, :])
    nc.sync.dma_start(ones_sb[:], ones[:, 0].rearrange("(a p) -> a p", a=1))

    return persist, dict(
        xT_sb=xT_sb, wq_sb=wq_sb, wk_sb=wk_sb, wv_sb=wv_sb, qT_sb=qT_sb,
        kT_sb=kT_sb, va_sb=va_sb, es_sb=es_sb, out_sb=out_sb, qb_sb=qb_sb,
        kb_sb=kb_sb, vb_sb=vb_sb, tri_sb=tri_sb, ones_sb=ones_sb)


def _compute(tc, t, out, dma_only=False):
    nc = tc.nc
    mask_eng = nc.gpsimd if MASK_ENGINE == "gpsimd" else nc.vector
    (xT_sb, wq_sb, wk_sb, wv_sb, qT_sb, kT_sb, va_sb, es_sb, out_sb, qb_sb,
     kb_sb, vb_sb, tri_sb, ones_sb) = (
        t["xT_sb"], t["wq_sb"], t["wk_sb"], t["wv_sb"], t["qT_sb"], t["kT_sb"],
        t["va_sb"], t["es_sb"], t["out_sb"], t["qb_sb"], t["kb_sb"], t["vb_sb"],
        t["tri_sb"], t["ones_sb"])
    small = tc.alloc_tile_pool(name="small", bufs=8)
    if dma_only:
        nc.sync.dma_start(out[0:P, 0:S], xT_sb[0][:].bitcast(F32))
        small.release()
        return

    # ---- fused projections + attention ------------------------------
    bp, bq, bv = BUFS
    assert bp + (2 if QK_WIDE else 1) * bq + bv <= 8
    dramp = tc.alloc_tile_pool(name="dramp", bufs=4, space="DRAM")
    with (
        tc.tile_pool(name="proj_psum", bufs=bp, space="PSUM") as pp,
        tc.tile_pool(name="qk_psum", bufs=bq, space="PSUM") as qkp,
        tc.tile_pool(name="pv_psum", bufs=bv, space="PSUM") as pvp,
    ):
        # warm the ACT exp table while input DMAs run
        warm = small.tile([1, 4], F32, tag="warm", name="warm")
        nc.scalar.activation(warm[:], ones_sb[0:1, 0:4].bitcast(F32),
                             mybir.ActivationFunctionType.Exp)

        def proj_qk_chunk(i):
            for (w_sb, dst, b_sb) in ((wq_sb, qT_sb, qb_sb), (wk_sb, kT_sb, kb_sb)):
                for half in range(2):
                    o = 512 * half
                    ps = pp.tile([P, 512], F32, tag="proj", name="proj")
                    for kc in range(CK):
                        nc.tensor.matmul(
                            ps[:], w_sb[kc][:, P * i:P * (i + 1)],
                            xT_sb[kc][:, o:o + 512],
                            start=(kc == 0), stop=(kc == CK - 1))
                    nc.vector.tensor_scalar_add(
                        dst[i][:, o:o + 512], ps[:], b_sb[:, i:i + 1])

        HW2 = NH * (HD + 1) // 2  # 260 aug-columns per half

        def proj_v_chunk(s):
            for half in range(2):
                o = HW2 * half
                ps = pp.tile([P, HW2], F32, tag="proj", name="proj")
                for kc in range(CK):
                    nc.tensor.matmul(
                        ps[:], xT_sb[kc][:, P * s:P * (s + 1)],
                        wv_sb[kc][:, o:o + HW2],
                        start=(kc == 0), stop=False)
                nc.tensor.matmul(
                    ps[:], ones_sb[0:1, :], vb_sb[0:1, o:o + HW2],
                    start=False, stop=True)
                nc.vector.tensor_copy(va_sb[s][:, o:o + HW2], ps[:])

        def evict(i, hp, rr, t):
            """normalize pv psum tile t -> out_sb slice -> DRAM (frees t)."""
            o = 512 * rr
            rc = small.tile([1, 512], F32, tag="recip", name="recip")
            nc.vector.reciprocal(rc[:], t[HD:HD + 1, :])
            bc = small.tile([HD, 512], F32, tag="bc", name="bc")
            if EVICT_BCAST == "dma":
                rd = dramp.tile([1, 512], F32, tag="rd", name="rd")
                nc.sync.dma_start(rd[:], rc[:])
                rd_b = bass.AP(tensor=rd.tensor, offset=rd.offset,
                               ap=[[0, HD]] + list(rd.ap)[1:])
                nc.sync.dma_start(bc[:], rd_b)
            else:
                nc.gpsimd.partition_broadcast(bc[:], rc[:])
            mul_eng = nc.gpsimd if EVICT_MUL == "gpsimd" else nc.vector
            mul_eng.tensor_mul(
                out_sb[i][HD * hp:HD * (hp + 1), o:o + 512],
                t[0:HD, :], bc[:])
            nc.sync.dma_start(
                out[P * i + HD * hp:P * i + HD * (hp + 1), o:o + 512],
                out_sb[i][HD * hp:HD * (hp + 1), o:o + 512])

        proj_qk_chunk(0)
        if not V_IN_PAIR0:
            for s in range(SK):
                proj_v_chunk(s)

        cmax = {0: CK * 2 - 1, 1: SK - 1}  # last ki-chunk per qi-range

        def emit_qk_exp(i, c):
            if STAGE < 2:
                return
            sc = P * c
            es = es_sb
            ranges = [rr for rr in range(2) if sc <= 512 * rr + 511]
            if QK_WIDE:
                qk = [qkp.tile([P, S], F32, tag="qk", name="qk")
                      for _ in range(2)]
                for rr in ranges:
                    o = 512 * rr
                    lo = max(o, sc)
                    for hp in range(2):
                        ro = HD * hp
                        nc.tensor.matmul(
                            qk[hp][:, lo:o + 512],
                            kT_sb[i][ro:ro + HD, sc:sc + P],
                            qT_sb[i][ro:ro + HD, lo:o + 512],
                            start=True, stop=True, tile_position=(ro, 0))
                for hp in range(2):
                    if STAGE < 3:
                        continue
                    nc.scalar.activation(
                        es[hp][c][:, sc:S], qk[hp][:, sc:S],
                        mybir.ActivationFunctionType.Exp, scale=0.125)
                    if STAGE >= 4:
                        mask_eng.tensor_mul(
                            es[hp][c][:, sc:sc + P],
                            es[hp][c][:, sc:sc + P], tri_sb[:])
            else:
                for rr in ranges:
                    o = 512 * rr
                    lo = max(o, sc)
                    qk = [qkp.tile([P, 512], F32, tag="qk", name="qk")
                          for _ in range(2)]
                    for hp in range(2):
                        ro = HD * hp
                        nc.tensor.matmul(
                            qk[hp][:, lo - o:512],
                            kT_sb[i][ro:ro + HD, sc:sc + P],
                            qT_sb[i][ro:ro + HD, lo:o + 512],
                            start=True, stop=True, tile_position=(ro, 0))
                    for hp in range(2):
                        if STAGE < 3:
                            continue
                        nc.scalar.activation(
                            es[hp][c][:, lo:o + 512],
                            qk[hp][:, lo - o:512],
                            mybir.ActivationFunctionType.Exp, scale=0.125)
                        if rr == c // 4 and STAGE >= 4:
                            mask_eng.tensor_mul(
                                es[hp][c][:, sc:sc + P],
                                es[hp][c][:, sc:sc + P], tri_sb[:])

        pv_tiles = {}

        def emit_pv(i, c):
            if STAGE < 5:
                return
            if i not in pv_tiles:
                pv_tiles[i] = [[pvp.tile([P, 512], F32, tag="pv", name="pv")
                                for _ in range(2)] for _ in range(2)]
            pv = pv_tiles[i]
            sc = P * c
            ranges = [rr for rr in range(2) if sc <= 512 * rr + 511]
            for rr in ranges:
                o = 512 * rr
                lo = max(o, sc)
                for hp in range(2):
                    h = 2 * i + hp
                    nc.tensor.matmul(
                        pv[hp][rr][0:HD + 1, lo - o:512],
                        va_sb[c][:, (HD + 1) * h:(HD + 1) * (h + 1)],
                        es_sb[hp][c][:, lo:o + 512],
                        start=(c == 0), stop=(c == cmax[rr]))
                if c == cmax[rr] and STAGE >= 6:
                    for hp in range(2):
                        evict(i, hp, rr, pv[hp][rr])

        LA = PV_LAG
        seq = [(i, c) for i in range(MK) for c in range(SK)]
        for idx, (i, c) in enumerate(seq):
            if i == 0 and V_IN_PAIR0:
                proj_v_chunk(c)  # stream v in during the first pair
            if c == 3 and i + 1 < MK:
                proj_qk_chunk(i + 1)
            emit_qk_exp(i, c)
            if idx >= LA:
                emit_pv(*seq[idx - LA])
        for idx in range(len(seq) - LA, len(seq)):
            emit_pv(*seq[idx])

    dramp.release()
    small.release()


_NC = None


def build_nc(reps=1, hw_loop=0):
    nc = bacc.Bacc("TRN2", target_bir_lowering=False, debug=False,
                   num_devices=NC_CORES)
    with tile.TileContext(nc) as tc:
        _emit(tc, reps=reps, hw_loop=hw_loop)
    nc.compile()
    return nc


def _get_nc():
    global _NC
    if _NC is None:
        _NC = build_nc()
    return _NC


def make_in_maps(input, q_w, q_b, k_w, k_b, v_w, v_b):
    xT_all = np.ascontiguousarray(input.reshape(B, C, S), dtype=np.float32)
    wqT = np.ascontiguousarray(q_w.T, dtype=np.float32)
    wkT = np.ascontiguousarray(k_w.T, dtype=np.float32)
    # v weights augmented with a zero column per head whose bias is 1.0 ->
    # the projection emits [v_h | 1] directly (softmax denominator column)
    wv = np.asarray(v_w, dtype=np.float32).reshape(NH, HD, C)
    wv_aug = np.concatenate([wv, np.zeros((NH, 1, C), np.float32)], axis=1)
    wvT = np.ascontiguousarray(wv_aug.reshape(NH * (HD + 1), C).T)
    vb_aug = np.concatenate(
        [np.asarray(v_b, np.float32).reshape(NH, HD),
         np.ones((NH, 1), np.float32)], axis=1).reshape(-1)
    tri = np.triu(np.ones((P, P), dtype=np.float32))  # 1 where ki<=qi
    if ATTN_BF16:
        import ml_dtypes
        tri = tri.astype(ml_dtypes.bfloat16)
    shared = {
        "wqT": wqT, "wkT": wkT, "wvT": wvT,
        "qb": np.ascontiguousarray(q_b, np.float32),
        "kb": np.ascontiguousarray(k_b, np.float32),
        "vb": vb_aug,
        "tri": tri,
        "ones": np.ones((P, NH + 1), dtype=np.float32),
    }
    return [{"xT": xT_all[b], **shared} for b in range(B)]


def kernel(input, q_w, q_b, k_w, k_b, v_w, v_b):
    nc = _get_nc()
    in_maps = make_in_maps(input, q_w, q_b, k_w, k_b, v_w, v_b)
    res = run_bass_kernel_spmd(nc, in_maps, core_ids=list(range(NC_CORES)))
    out = np.stack([res.results[b]["out"] for b in range(B)])
    return out.reshape(B, KD, H, W)

